# revision 3
# baseline (speedup 1.0000x reference)
"""2-layer GCN (GCNConv -> ReLU -> GCNConv) on 8 TRN2 NeuronCores.

Sharding: output nodes are split into 8 shards (one per core); edges are
partitioned by destination shard so each core owns the scatter-add for its
nodes. Hidden features of source nodes are exchanged with an on-device
AllGather between the per-shard transform and the aggregation.

Per-core pipeline (single SPMD Bass program, all cores identical; per-core
behavior comes from per-core input data):
  1. transform: h' = dinv * (x @ W1) for the own shard (TensorE, bf16
     operands / fp32 PSUM accumulate), host-pretransposed bf16 x.
  2. AllGather h' (bf16) -> full 50176-row table in each core's DRAM.
  3. aggregation: edges are pre-sorted by destination block (49 blocks of
     128 dst nodes per core, balanced by in-degree via a host-side node
     relabeling). Source rows are fetched with dma_gather (256B bf16 rows).
     dma_gather indices are int16, so edges are segregated into a "lo"
     (src < 32768) stream and a "hi" stream addressed from a shifted table
     base. Each stream is a flat sequence of 128-edge chunks (per dst
     block: lo chunks then hi chunks, zero-padded to chunk granularity);
     gather calls cover up to CALL_CHUNKS consecutive chunks regardless of
     block boundaries, so the 994ns/call SWDGE desc-gen overhead amortizes
     over 4096 indices (needs dynamic_dma_scratch_size=64KB: the SWDGE
     ring carveout is scratch/16 descriptors per queue). lo calls ride
     queues 0/1, hi calls queues 2/3. A one-hot matrix S (tensor_scalar
     iota == dmat column, bf16 so DVE runs in 2x mode) folds each chunk
     into the block's PSUM accumulator on TensorE (bf16).
  4. out = dinv * (agg + h'_own) + bias (the self-loop and symmetric
     normalization fold into two dinv scalings); ReLU on ScalarE.
  5. repeat 1-4 with W2/b2; z shard is DMA'd out and un-permuted on host.

Gather prefetch: the first gather call of each queue per layer is
desc-gen'd with prepare_only=True on the otherwise idle Q7 before the
AllGather and fired by per-queue trigger_dma right after it. The triggers
carry explicit sync deps on the collective (the deferred RAW cannot see a
writer emitted after the prep), and per-queue gpsimd memset "gate"
instructions carrying _wait_ge on the prep DMA semaphores give the prepped
calls' fold chains a hardware data gate (Tile's own consumer dep resolves
on the prep's desc-written tick, not DMA completion).
"""

import os

import numpy as np

P = 128
N_CORES = 8
N_NODES = 50000
IN_DIM = 256
HID = 128
NB = 49
SHARD = NB * P  # 6272
NPAD = N_CORES * SHARD  # 50176
HI_BASE = 32768
CALL_CHUNKS = 8  # 1024 indices per gather call (isolate-call-size test)
DMA_SCRATCH = 65536  # SWDGE ring carveout: 4096 descriptors per queue
GLO_BUFS = 3  # lo-stream call-tile pool depth
GHI_BUFS = 2  # hi-stream call-tile pool depth

LAST_EXEC_NS = None
LAST_RESULT = None


def _wrap16(flat, ncols):
    w = np.zeros((16, ncols), np.uint16)
    n = len(flat)
    w[np.arange(n) % 16, np.arange(n) // 16] = flat
    return np.tile(w, (8, 1)).view(np.int16)


def _host_prep(x, edge_index, W1, b1, W2, b2):
    src = np.asarray(edge_index[0], dtype=np.int64)
    dst = np.asarray(edge_index[1], dtype=np.int64)
    x = np.asarray(x, dtype=np.float32)

    indeg = np.bincount(dst, minlength=N_NODES)
    deg = indeg + 1.0
    dinv = (1.0 / np.sqrt(deg)).astype(np.float32)

    # per-shard relabeling: deal nodes (by in-degree desc) round-robin into
    # the 49 dst blocks so block edge counts are balanced across cores.
    old_shard = N_NODES // N_CORES
    new_of_old = np.empty(N_NODES, np.int64)
    old_of_new = np.full(NPAD, -1, np.int64)
    for c in range(N_CORES):
        olds = np.arange(c * old_shard, (c + 1) * old_shard)
        order = olds[np.argsort(-indeg[olds], kind="stable")]
        pos_in_block = np.arange(len(order)) // NB
        block = np.arange(len(order)) % NB
        news = c * SHARD + block * P + pos_in_block
        new_of_old[order] = news
        old_of_new[news] = order

    src_n = new_of_old[src]
    dst_n = new_of_old[dst]

    core_of_dst = dst_n // SHARD
    lo_lists = [[None] * NB for _ in range(N_CORES)]
    hi_lists = [[None] * NB for _ in range(N_CORES)]
    for c in range(N_CORES):
        m = core_of_dst == c
        s, d = src_n[m], dst_n[m] - c * SHARD
        b = d // P
        r = d % P
        hi = s >= HI_BASE
        for bb in range(NB):
            mb = b == bb
            mlo = mb & ~hi
            mhi = mb & hi
            lo_lists[c][bb] = (s[mlo], r[mlo])
            hi_lists[c][bb] = (s[mhi] - HI_BASE, r[mhi])

    # common (max-over-cores) chunk counts per block and region
    C_lo = np.zeros(NB, np.int64)
    C_hi = np.zeros(NB, np.int64)
    for b in range(NB):
        for c in range(N_CORES):
            C_lo[b] = max(C_lo[b], (len(lo_lists[c][b][0]) + P - 1) // P)
            C_hi[b] = max(C_hi[b], (len(hi_lists[c][b][0]) + P - 1) // P)
    lo0 = np.concatenate([[0], np.cumsum(C_lo)])  # [NB+1]
    hi0 = np.concatenate([[0], np.cumsum(C_hi)])
    NLO = int(lo0[-1])
    NHI = int(hi0[-1])
    NS = NLO + NHI

    # per-core flat index stream ([lo chunks][hi chunks]) + dst-row matrix
    idx_mats, d_mats = [], []
    for c in range(N_CORES):
        idx_flat = np.zeros(NS * P, np.int64)
        dloc = np.full((P, NS), -1.0, np.float32)
        for b in range(NB):
            for lists, c0 in ((lo_lists, lo0[b]), (hi_lists, NLO + hi0[b])):
                s, r = lists[c][b]
                n = len(s)
                base = int(c0) * P
                idx_flat[base : base + n] = s
                j = np.arange(n)
                dloc[j % P, int(c0) + j // P] = r
        idx_mats.append(_wrap16(idx_flat, NS * 8))
        d_mats.append(dloc)

    def mk_calls(total):
        calls = []
        at = 0
        while at < total:
            k = min(CALL_CHUNKS, total - at)
            calls.append((at, k))
            at += k
        return calls

    calls_lo = mk_calls(NLO)
    calls_hi = mk_calls(NHI)

    xs, dinvs = [], []
    for c in range(N_CORES):
        xc = np.zeros((SHARD, IN_DIM), np.float32)
        dc = np.ones((SHARD,), np.float32)
        sel = old_of_new[c * SHARD : (c + 1) * SHARD]
        real = sel >= 0
        xc[real] = x[sel[real]]
        dc[real] = dinv[sel[real]]
        dw = dc.reshape(NB, P).T.copy()
        xT = np.ascontiguousarray(xc.T.reshape(2, P, SHARD).transpose(1, 0, 2))
        xs.append(xT.reshape(P, 2 * SHARD))
        dinvs.append(dw)

    import ml_dtypes

    bf16 = ml_dtypes.bfloat16
    iota = np.tile(np.arange(P, dtype=np.float32)[None, :], (P, 1))
    ident = np.eye(P, dtype=np.float32).astype(bf16)
    b1r = np.tile(np.asarray(b1, np.float32)[None, :], (P, 1))
    b2r = np.tile(np.asarray(b2, np.float32)[None, :], (P, 1))

    in_maps = []
    for c in range(N_CORES):
        in_maps.append(
            {
                "x": xs[c].astype(bf16),
                "gidx": idx_mats[c],
                "dmat": d_mats[c],
                "dinv": dinvs[c],
                "w1": np.asarray(W1, np.float32).astype(bf16),
                "w2": np.asarray(W2, np.float32).astype(bf16),
                "b1r": b1r,
                "b2r": b2r,
                "iota": iota.astype(bf16),
                "ident": ident,
            }
        )

    meta = dict(
        C_lo=C_lo, C_hi=C_hi, lo0=lo0, hi0=hi0, NLO=NLO, NHI=NHI, NS=NS,
        calls_lo=calls_lo, calls_hi=calls_hi, old_of_new=old_of_new,
    )
    return in_maps, meta


NQ = 4  # SWDGE queues: lo stream on 0/1, hi stream on 2/3


def _build_program(meta):
    import concourse.mybir as mybir
    import concourse.tile as tile
    from concourse import bacc
    from concourse._compat import get_trn_type
    from concourse.instruction_name_ordered_set import InstructionNameOrderedSet

    C_lo, C_hi = meta["C_lo"], meta["C_hi"]
    lo0, hi0 = meta["lo0"], meta["hi0"]
    NLO, NHI, NS = meta["NLO"], meta["NHI"], meta["NS"]
    calls_lo, calls_hi = meta["calls_lo"], meta["calls_hi"]
    f32 = mybir.dt.float32
    bf = mybir.dt.bfloat16
    tdt = bf

    nc = bacc.Bacc(
        get_trn_type() or "TRN2",
        num_swdge_queues=NQ,
        dynamic_dma_scratch_size=DMA_SCRATCH,
    )
    x_in = nc.dram_tensor("x", [P, 2 * SHARD], bf, kind="ExternalInput")
    gidx = nc.dram_tensor("gidx", [P, NS * 8], mybir.dt.int16, kind="ExternalInput")
    dmat = nc.dram_tensor("dmat", [P, NS], f32, kind="ExternalInput")
    dinv_in = nc.dram_tensor("dinv", [P, NB], f32, kind="ExternalInput")
    w1_in = nc.dram_tensor("w1", [IN_DIM, HID], bf, kind="ExternalInput")
    w2_in = nc.dram_tensor("w2", [HID, HID], bf, kind="ExternalInput")
    b1_in = nc.dram_tensor("b1r", [P, HID], f32, kind="ExternalInput")
    b2_in = nc.dram_tensor("b2r", [P, HID], f32, kind="ExternalInput")
    iota_in = nc.dram_tensor("iota", [P, P], bf, kind="ExternalInput")
    ident_in = nc.dram_tensor("ident", [P, P], bf, kind="ExternalInput")
    z_out = nc.dram_tensor("z", [SHARD, HID], f32, kind="ExternalOutput")

    cc1_in = nc.dram_tensor("cc1_in", [SHARD, HID], tdt)
    table1 = nc.dram_tensor("table1", [NPAD, HID], tdt, addr_space="Shared")
    cc2_in = nc.dram_tensor("cc2_in", [SHARD, HID], tdt)
    table2 = nc.dram_tensor("table2", [NPAD, HID], tdt, addr_space="Shared")

    rg = [list(range(N_CORES))]
    dma_sems = [nc.alloc_semaphore(f"gdma{q}") for q in range(NQ)]

    with tile.TileContext(nc) as tc:
        for _s in dma_sems:
            nc.gpsimd.sem_clear(_s)
        with (
            tc.tile_pool(name="persist", bufs=1) as pp,
            tc.tile_pool(name="xt", bufs=4) as xtp,
            tc.tile_pool(name="glo", bufs=GLO_BUFS) as gplo,
            tc.tile_pool(name="ghi", bufs=GHI_BUFS) as gphi,
            tc.tile_pool(name="s", bufs=8) as sp,
            tc.tile_pool(name="ep", bufs=4) as ep,
            tc.tile_pool(name="psum", bufs=2, space="PSUM") as psp,
        ):
            idx_t = pp.tile([P, NS * 8], mybir.dt.int16)
            nc.sync.dma_start(out=idx_t[:], in_=gidx[:])
            dm_t = pp.tile([P, NS], f32)
            nc.sync.dma_start(out=dm_t[:], in_=dmat[:])
            dinv_t = pp.tile([P, NB], f32)
            nc.sync.dma_start(out=dinv_t[:], in_=dinv_in[:])
            iota_t = pp.tile([P, P], bf)
            nc.sync.dma_start(out=iota_t[:], in_=iota_in[:])
            ident_t = pp.tile([P, P], bf)
            nc.sync.dma_start(out=ident_t[:], in_=ident_in[:])
            b1_t = pp.tile([P, HID], f32)
            nc.sync.dma_start(out=b1_t[:], in_=b1_in[:])
            b2_t = pp.tile([P, HID], f32)
            nc.sync.dma_start(out=b2_t[:], in_=b2_in[:])
            w1_t = pp.tile([P, 2 * HID], bf)
            nc.sync.dma_start(
                out=w1_t[:].rearrange("p (k h) -> p k h", k=2),
                in_=w1_in[:].rearrange("(k p) h -> p k h", p=P),
            )
            w2_t = pp.tile([P, HID], bf)
            nc.sync.dma_start(out=w2_t[:], in_=w2_in[:])

            hbuf = pp.tile([P, SHARD], tdt)
            x2buf = pp.tile([P, SHARD], bf)
            h2buf = pp.tile([P, SHARD], tdt)

            def transform(get_lhsT, w_tiles, out_sbuf, cc_dram):
                nkt = len(w_tiles)
                for t in range(NB):
                    hp = psp.tile([P, HID], f32, tag="hp")
                    for k in range(nkt):
                        nc.tensor.matmul(
                            out=hp[:], lhsT=get_lhsT(t, k), rhs=w_tiles[k],
                            start=(k == 0), stop=(k == nkt - 1),
                        )
                    sl = out_sbuf[:, t * P : (t + 1) * P]
                    nc.vector.tensor_scalar(
                        out=sl, in0=hp[:], scalar1=dinv_t[:, t : t + 1],
                        scalar2=None, op0=mybir.AluOpType.mult,
                    )
                    nc.sync.dma_start(
                        out=cc_dram[t * P : (t + 1) * P, :], in_=sl
                    )

            prep_counts = [0] * NQ

            def aggregate(table, hsrc, bias_t, relu, z_dram, do_collective):
                # call tiles (per stream), chunk-indexed views
                lo_tiles = {}
                hi_tiles = {}

                def issue(stream, ci, prep):
                    calls, tiles, gp, qbase = (
                        (calls_lo, lo_tiles, gplo, 0)
                        if stream == "lo"
                        else (calls_hi, hi_tiles, gphi, 2)
                    )
                    at, k = calls[ci]
                    n = k * P
                    src = table[HI_BASE:, :] if stream == "hi" else table[:, :]
                    col = at if stream == "lo" else NLO + at
                    q = qbase + ci % 2
                    kw = (
                        dict(prepare_only=True, sem=dma_sems[q]) if prep else {}
                    )
                    G = gp.tile([P, CALL_CHUNKS * HID], tdt, tag=f"g{stream}")
                    G3 = G[:].rearrange("p (c d) -> p c d", d=HID)
                    tiles[ci] = G3
                    nc.gpsimd.dma_gather(
                        G3[:, 0:k, :],
                        src,
                        idx_t[:, col * 8 : col * 8 + n // 16],
                        n, n, HID,
                        queue_num=q,
                        **kw,
                    )
                    return q

                # prep the first call of each queue (ring holds one
                # 4096-desc call per queue)
                npre_lo = min(2, len(calls_lo))
                npre_hi = min(2, len(calls_hi))
                prep_q = {}
                for ci in range(npre_lo):
                    q = issue("lo", ci, True)
                    prep_counts[q] += 1
                    prep_q[("lo", ci)] = q
                for ci in range(npre_hi):
                    q = issue("hi", ci, True)
                    prep_counts[q] += 1
                    prep_q[("hi", ci)] = q

                ag = do_collective()
                ag_dep = InstructionNameOrderedSet()
                ag_dep.add(ag.ins.name)
                qs_used = sorted(set(prep_q.values()))
                trig_dep = InstructionNameOrderedSet()
                for q in qs_used:
                    trig = nc.gpsimd.trigger_dma(count=None, queue_num=q)
                    trig.ins.add_sync_dependencies_from(ag_dep)
                    trig_dep.add(trig.ins.name)
                gate_deps = {}
                for q in qs_used:
                    gate_t = ep.tile([P, 4], f32, tag="gate")
                    g_i = nc.gpsimd.memset(gate_t[:], 0.0)
                    g_i._wait_ge(dma_sems[q], 16 * prep_counts[q])
                    g_i.ins.add_nosync_dependencies_from(trig_dep)
                    d = InstructionNameOrderedSet()
                    d.add(g_i.ins.name)
                    gate_deps[q] = d
                ungated = {k: prep_q[k] for k in prep_q}

                li = npre_lo
                hj = npre_hi
                LOOKAHEAD = CALL_CHUNKS  # issue one extra call ahead

                for b in range(NB):
                    lo_end = int(lo0[b]) + int(C_lo[b])
                    hi_end = int(hi0[b]) + int(C_hi[b])
                    while li < len(calls_lo) and calls_lo[li][0] < lo_end + LOOKAHEAD:
                        issue("lo", li, False)
                        li += 1
                    while hj < len(calls_hi) and calls_hi[hj][0] < hi_end + LOOKAHEAD:
                        issue("hi", hj, False)
                        hj += 1
                    acc = psp.tile([P, HID], f32, tag="acc")
                    nfold = int(C_lo[b]) + int(C_hi[b])
                    fi = 0
                    for stream, c0, cnt, coff, tiles in (
                        ("lo", int(lo0[b]), int(C_lo[b]), 0, lo_tiles),
                        ("hi", int(hi0[b]), int(C_hi[b]), NLO, hi_tiles),
                    ):
                        for i in range(cnt):
                            ch = c0 + i  # stream-local chunk index
                            col = coff + ch  # dmat column
                            ci = ch // CALL_CHUNKS
                            slot = ch % CALL_CHUNKS
                            S = sp.tile([P, P], tdt, tag="S")
                            nc.vector.tensor_scalar(
                                out=S[:], in0=iota_t[:],
                                scalar1=dm_t[:, col : col + 1],
                                scalar2=None,
                                op0=mybir.AluOpType.is_equal,
                            )
                            mm = nc.tensor.matmul(
                                out=acc[:], lhsT=S[:],
                                rhs=tiles[ci][:, slot, :],
                                start=(fi == 0), stop=(fi == nfold - 1),
                            )
                            key = (stream, ci)
                            if key in ungated:
                                mm.ins.add_sync_dependencies_from(
                                    gate_deps[ungated.pop(key)]
                                )
                            fi += 1
                    t1 = ep.tile([P, HID], f32, tag="t1")
                    nc.vector.tensor_tensor(
                        out=t1[:], in0=acc[:],
                        in1=hsrc[:, b * P : (b + 1) * P],
                        op=mybir.AluOpType.add,
                    )
                    t2 = ep.tile([P, HID], f32, tag="t2")
                    nc.vector.scalar_tensor_tensor(
                        out=t2[:], in0=t1[:],
                        scalar=dinv_t[:, b : b + 1], in1=bias_t[:],
                        op0=mybir.AluOpType.mult, op1=mybir.AluOpType.add,
                    )
                    if relu:
                        nc.scalar.activation(
                            out=x2buf[:, b * P : (b + 1) * P], in_=t2[:],
                            func=mybir.ActivationFunctionType.Relu,
                        )
                    else:
                        nc.sync.dma_start(
                            out=z_dram[b * P : (b + 1) * P, :], in_=t2[:]
                        )

            # layer 1
            xT_t = pp.tile([P, 2 * SHARD], bf)
            nc.sync.dma_start(out=xT_t[:], in_=x_in[:])
            xT3 = xT_t[:].rearrange("p (k n) -> p k n", k=2)
            transform(
                lambda t, k: xT3[:, k, t * P : (t + 1) * P],
                [w1_t[:, 0:HID], w1_t[:, HID : 2 * HID]], hbuf, cc1_in,
            )
            aggregate(
                table1, hbuf, b1_t, True, None,
                lambda: nc.gpsimd.collective_compute(
                    "AllGather", mybir.AluOpType.bypass, replica_groups=rg,
                    ins=[cc1_in[:]], outs=[table1[:]],
                ),
            )

            # layer 2
            def l2_lhsT(t, k):
                tp = psp.tile([P, P], bf, tag="tp")
                nc.tensor.transpose(
                    out=tp[:], in_=x2buf[:, t * P : (t + 1) * P],
                    identity=ident_t[:],
                )
                xT = xtp.tile([P, P], bf, tag="xT")
                nc.scalar.copy(out=xT[:], in_=tp[:])
                return xT[:]

            transform(l2_lhsT, [w2_t[:]], h2buf, cc2_in)
            aggregate(
                table2, h2buf, b2_t, False, z_out,
                lambda: nc.gpsimd.collective_compute(
                    "AllGather", mybir.AluOpType.bypass, replica_groups=rg,
                    ins=[cc2_in[:]], outs=[table2[:]],
                ),
            )

    nc.compile()
    return nc


def kernel(x, edge_index, W1, b1, W2, b2):
    global LAST_EXEC_NS, LAST_RESULT
    from concourse.bass_utils import run_bass_kernel_spmd

    trace = bool(int(os.environ.get("GCN_TRACE", "0")))
    if trace:
        try:  # NTFF profiling shim (axon images lack antenv.axon_hooks)
            _install_ntff_shim()
        except Exception:
            trace = False

    in_maps, meta = _host_prep(x, edge_index, W1, b1, W2, b2)
    nc = _build_program(meta)
    res = run_bass_kernel_spmd(
        nc, in_maps, core_ids=list(range(N_CORES)), trace=trace
    )
    LAST_EXEC_NS = res.exec_time_ns
    LAST_RESULT = res

    old_of_new = meta["old_of_new"]
    z = np.zeros((N_NODES, HID), np.float32)
    for c in range(N_CORES):
        zc = np.asarray(res.results[c]["z"])
        sel = old_of_new[c * SHARD : (c + 1) * SHARD]
        real = sel >= 0
        z[sel[real]] = zc[real]
    return z


def _install_ntff_shim():
    import contextlib
    import ctypes
    import sys
    import types

    if "antenv.axon_hooks" in sys.modules:
        return
    lib = ctypes.CDLL("/opt/axon/libaxon_pjrt.so")
    if not hasattr(lib, "axon_start_nrt_profile"):
        raise RuntimeError("no profile symbols")
    lib.axon_start_nrt_profile.argtypes = [
        ctypes.POINTER(ctypes.c_int64),
        ctypes.c_size_t,
    ]
    lib.axon_start_nrt_profile.restype = ctypes.c_int64
    lib.axon_stop_nrt_profile.argtypes = [ctypes.c_char_p]
    lib.axon_stop_nrt_profile.restype = ctypes.c_int64

    @contextlib.contextmanager
    def _hook(output_dir, device_ids):
        import jax

        jax.devices()
        if device_ids:
            ids = (ctypes.c_int64 * len(device_ids))(*device_ids)
            rc = lib.axon_start_nrt_profile(ids, len(device_ids))
        else:
            rc = lib.axon_start_nrt_profile(None, 0)
        if rc != 0:
            raise RuntimeError(f"axon_start_nrt_profile rc={rc}")
        try:
            yield
        finally:
            lib.axon_stop_nrt_profile(str(output_dir).encode())

    mod = types.ModuleType("antenv.axon_hooks")
    mod.get_axon_ntff_profile_hook = lambda: _hook
    mod.set_axon_ntff_profile_hook = lambda h: None
    sys.modules["antenv.axon_hooks"] = mod
    import antenv

    antenv.axon_hooks = mod


# revision 15
# speedup vs baseline: 1.2055x; 1.2055x over previous
"""2-layer GCN (GCNConv -> ReLU -> GCNConv) on 8 TRN2 NeuronCores.

Sharding: output nodes are split into 8 shards (one per core); edges are
partitioned by destination shard so each core owns the scatter-add for its
nodes. Hidden features of source nodes are exchanged with an on-device
AllGather between the per-shard transform and the aggregation.

Per-core pipeline (single SPMD Bass program, all cores identical; per-core
behavior comes from per-core input data):
  1. transform: h' = dinv * (x @ W1) for the own shard (TensorE, bf16
     operands / fp32 PSUM accumulate), host-pretransposed bf16 x.
  2. AllGather h' (bf16) -> full 50176-row table in each core's DRAM.
  3. aggregation: edges are pre-sorted by destination block (49 blocks of
     128 dst nodes per core, balanced by in-degree via a host-side node
     relabeling). Source rows are fetched with dma_gather (256B bf16 rows).
     dma_gather indices are int16, so edges are segregated into a "lo"
     (src < 32768) stream and a "hi" stream addressed from a shifted table
     base. Each stream is a flat sequence of 128-edge chunks (per dst
     block: lo chunks then hi chunks, zero-padded to chunk granularity);
     gather calls cover up to CALL_CHUNKS consecutive chunks regardless of
     block boundaries, so the 994ns/call SWDGE desc-gen overhead amortizes
     over 4096 indices (needs dynamic_dma_scratch_size=64KB: the SWDGE
     ring carveout is scratch/16 descriptors per queue). lo calls ride
     queues 0/1, hi calls queues 2/3. A one-hot matrix S (tensor_scalar
     iota == dmat column, bf16 so DVE runs in 2x mode) folds each chunk
     into the block's PSUM accumulator on TensorE (bf16).
  4. out = dinv * (agg + h'_own) + bias (the self-loop and symmetric
     normalization fold into two dinv scalings); ReLU on ScalarE.
  5. repeat 1-4 with W2/b2; z shard is DMA'd out and un-permuted on host.

Gather prefetch: the first gather call of each queue per layer is
desc-gen'd with prepare_only=True on the otherwise idle Q7 before the
AllGather and fired by per-queue trigger_dma right after it. The triggers
carry explicit sync deps on the collective (the deferred RAW cannot see a
writer emitted after the prep), and per-queue gpsimd memset "gate"
instructions carrying _wait_ge on the prep DMA semaphores give the prepped
calls' fold chains a hardware data gate (Tile's own consumer dep resolves
on the prep's desc-written tick, not DMA completion).
"""

import os

import numpy as np

P = 128
N_CORES = 8
N_NODES = 50000
IN_DIM = 256
HID = 128
NB = 49
SHARD = NB * P  # 6272
NPAD = N_CORES * SHARD  # 50176
HI_BASE = 32768
CALL_CHUNKS = 8  # 1024 indices per gather call (hard SWDGE ucode limit)
DMA_SCRATCH = 65536  # SWDGE ring carveout: 4096 descriptors per queue
GLO_BUFS = 12  # lo-stream call-tile pool depth
GHI_BUFS = 8  # hi-stream call-tile pool depth

LAST_EXEC_NS = None
LAST_RESULT = None


def _wrap16(flat, ncols):
    w = np.zeros((16, ncols), np.uint16)
    n = len(flat)
    w[np.arange(n) % 16, np.arange(n) // 16] = flat
    return np.tile(w, (8, 1)).view(np.int16)


def _host_prep(x, edge_index, W1, b1, W2, b2):
    src = np.asarray(edge_index[0], dtype=np.int64)
    dst = np.asarray(edge_index[1], dtype=np.int64)
    x = np.asarray(x, dtype=np.float32)

    indeg = np.bincount(dst, minlength=N_NODES)
    deg = indeg + 1.0
    dinv = (1.0 / np.sqrt(deg)).astype(np.float32)

    # per-shard relabeling: deal nodes (by in-degree desc) round-robin into
    # the 49 dst blocks so block edge counts are balanced across cores.
    old_shard = N_NODES // N_CORES
    new_of_old = np.empty(N_NODES, np.int64)
    old_of_new = np.full(NPAD, -1, np.int64)
    for c in range(N_CORES):
        olds = np.arange(c * old_shard, (c + 1) * old_shard)
        order = olds[np.argsort(-indeg[olds], kind="stable")]
        pos_in_block = np.arange(len(order)) // NB
        block = np.arange(len(order)) % NB
        news = c * SHARD + block * P + pos_in_block
        new_of_old[order] = news
        old_of_new[news] = order

    src_n = new_of_old[src]
    dst_n = new_of_old[dst]

    core_of_dst = dst_n // SHARD
    lo_lists = [[None] * NB for _ in range(N_CORES)]
    hi_lists = [[None] * NB for _ in range(N_CORES)]
    for c in range(N_CORES):
        m = core_of_dst == c
        s, d = src_n[m], dst_n[m] - c * SHARD
        b = d // P
        r = d % P
        hi = s >= HI_BASE
        for bb in range(NB):
            mb = b == bb
            mlo = mb & ~hi
            mhi = mb & hi
            lo_lists[c][bb] = (s[mlo], r[mlo])
            hi_lists[c][bb] = (s[mhi] - HI_BASE, r[mhi])

    # common (max-over-cores) chunk counts per block and region
    C_lo = np.zeros(NB, np.int64)
    C_hi = np.zeros(NB, np.int64)
    for b in range(NB):
        for c in range(N_CORES):
            C_lo[b] = max(C_lo[b], (len(lo_lists[c][b][0]) + P - 1) // P)
            C_hi[b] = max(C_hi[b], (len(hi_lists[c][b][0]) + P - 1) // P)
    lo0 = np.concatenate([[0], np.cumsum(C_lo)])  # [NB+1]
    hi0 = np.concatenate([[0], np.cumsum(C_hi)])
    NLO = int(lo0[-1])
    NHI = int(hi0[-1])
    NS = NLO + NHI

    # per-core flat index stream ([lo chunks][hi chunks]) + dst-row matrix
    idx_mats, d_mats = [], []
    for c in range(N_CORES):
        idx_flat = np.zeros(NS * P, np.int64)
        dloc = np.full((P, NS), -1.0, np.float32)
        for b in range(NB):
            for lists, c0 in ((lo_lists, lo0[b]), (hi_lists, NLO + hi0[b])):
                s, r = lists[c][b]
                n = len(s)
                base = int(c0) * P
                idx_flat[base : base + n] = s
                j = np.arange(n)
                dloc[j % P, int(c0) + j // P] = r
        idx_mats.append(_wrap16(idx_flat, NS * 8))
        d_mats.append(dloc)

    def mk_calls(total):
        calls = []
        at = 0
        while at < total:
            k = min(CALL_CHUNKS, total - at)
            calls.append((at, k))
            at += k
        return calls

    calls_lo = mk_calls(NLO)
    calls_hi = mk_calls(NHI)

    xs, dinvs = [], []
    for c in range(N_CORES):
        xc = np.zeros((SHARD, IN_DIM), np.float32)
        dc = np.ones((SHARD,), np.float32)
        sel = old_of_new[c * SHARD : (c + 1) * SHARD]
        real = sel >= 0
        xc[real] = x[sel[real]]
        dc[real] = dinv[sel[real]]
        dw = dc.reshape(NB, P).T.copy()
        xT = np.ascontiguousarray(xc.T.reshape(2, P, SHARD).transpose(1, 0, 2))
        xs.append(xT.reshape(P, 2 * SHARD))
        dinvs.append(dw)

    import ml_dtypes

    bf16 = ml_dtypes.bfloat16
    iota = np.tile(np.arange(P, dtype=np.float32)[None, :], (P, 1))
    ident = np.eye(P, dtype=np.float32).astype(bf16)
    b1r = np.tile(np.asarray(b1, np.float32)[None, :], (P, 1))
    b2r = np.tile(np.asarray(b2, np.float32)[None, :], (P, 1))

    in_maps = []
    for c in range(N_CORES):
        in_maps.append(
            {
                "x": xs[c].astype(bf16),
                "gidx": idx_mats[c],
                "dmat": d_mats[c],
                "dinv": dinvs[c],
                "w1": np.asarray(W1, np.float32).astype(bf16),
                "w2": np.asarray(W2, np.float32).astype(bf16),
                "b1r": b1r,
                "b2r": b2r,
                "iota": iota.astype(bf16),
                "ident": ident,
            }
        )

    meta = dict(
        C_lo=C_lo, C_hi=C_hi, lo0=lo0, hi0=hi0, NLO=NLO, NHI=NHI, NS=NS,
        calls_lo=calls_lo, calls_hi=calls_hi, old_of_new=old_of_new,
        has_bias=bool(np.any(np.asarray(b1)) or np.any(np.asarray(b2))),
    )
    return in_maps, meta


NQ = 4  # SWDGE queues: lo stream on 0/1, hi stream on 2/3


def _build_program(meta):
    import concourse.mybir as mybir
    import concourse.tile as tile
    from concourse import bacc
    from concourse._compat import get_trn_type
    from concourse.instruction_name_ordered_set import InstructionNameOrderedSet

    C_lo, C_hi = meta["C_lo"], meta["C_hi"]
    has_bias = meta["has_bias"]
    lo0, hi0 = meta["lo0"], meta["hi0"]
    NLO, NHI, NS = meta["NLO"], meta["NHI"], meta["NS"]
    calls_lo, calls_hi = meta["calls_lo"], meta["calls_hi"]
    f32 = mybir.dt.float32
    bf = mybir.dt.bfloat16
    tdt = bf

    nc = bacc.Bacc(
        get_trn_type() or "TRN2",
        num_swdge_queues=NQ,
        dynamic_dma_scratch_size=DMA_SCRATCH,
    )
    x_in = nc.dram_tensor("x", [P, 2 * SHARD], bf, kind="ExternalInput")
    gidx = nc.dram_tensor("gidx", [P, NS * 8], mybir.dt.int16, kind="ExternalInput")
    dmat = nc.dram_tensor("dmat", [P, NS], f32, kind="ExternalInput")
    dinv_in = nc.dram_tensor("dinv", [P, NB], f32, kind="ExternalInput")
    w1_in = nc.dram_tensor("w1", [IN_DIM, HID], bf, kind="ExternalInput")
    w2_in = nc.dram_tensor("w2", [HID, HID], bf, kind="ExternalInput")
    b1_in = nc.dram_tensor("b1r", [P, HID], f32, kind="ExternalInput")
    b2_in = nc.dram_tensor("b2r", [P, HID], f32, kind="ExternalInput")
    iota_in = nc.dram_tensor("iota", [P, P], bf, kind="ExternalInput")
    ident_in = nc.dram_tensor("ident", [P, P], bf, kind="ExternalInput")
    z_out = nc.dram_tensor("z", [SHARD, HID], f32, kind="ExternalOutput")

    cc1_in = nc.dram_tensor("cc1_in", [SHARD, HID], tdt)
    table1 = nc.dram_tensor("table1", [NPAD, HID], tdt, addr_space="Shared")
    cc2_in = nc.dram_tensor("cc2_in", [SHARD, HID], tdt)
    table2 = nc.dram_tensor("table2", [NPAD, HID], tdt, addr_space="Shared")

    rg = [list(range(N_CORES))]
    dma_sems = [nc.alloc_semaphore(f"gdma{q}") for q in range(NQ)]

    with tile.TileContext(nc) as tc:
        for _s in dma_sems:
            nc.gpsimd.sem_clear(_s)
        with (
            tc.tile_pool(name="persist", bufs=1) as pp,
            tc.tile_pool(name="xt", bufs=4) as xtp,
            tc.tile_pool(name="glo", bufs=GLO_BUFS) as gplo,
            tc.tile_pool(name="ghi", bufs=GHI_BUFS) as gphi,
            tc.tile_pool(name="s", bufs=8) as sp,
            tc.tile_pool(name="ep", bufs=4) as ep,
            tc.tile_pool(name="psum", bufs=2, space="PSUM") as psp,
        ):
            idx_t = pp.tile([P, NS * 8], mybir.dt.int16)
            nc.sync.dma_start(out=idx_t[:], in_=gidx[:])
            dm_t = pp.tile([P, NS], f32)
            nc.sync.dma_start(out=dm_t[:], in_=dmat[:])
            dinv_t = pp.tile([P, NB], f32)
            nc.sync.dma_start(out=dinv_t[:], in_=dinv_in[:])
            iota_t = pp.tile([P, P], bf)
            nc.sync.dma_start(out=iota_t[:], in_=iota_in[:])
            ident_t = pp.tile([P, P], bf)
            nc.sync.dma_start(out=ident_t[:], in_=ident_in[:])
            b1_t = pp.tile([P, HID], f32)
            nc.sync.dma_start(out=b1_t[:], in_=b1_in[:])
            b2_t = pp.tile([P, HID], f32)
            nc.sync.dma_start(out=b2_t[:], in_=b2_in[:])
            w1_t = pp.tile([P, 2 * HID], bf)
            nc.sync.dma_start(
                out=w1_t[:].rearrange("p (k h) -> p k h", k=2),
                in_=w1_in[:].rearrange("(k p) h -> p k h", p=P),
            )
            w2_t = pp.tile([P, HID], bf)
            nc.sync.dma_start(out=w2_t[:], in_=w2_in[:])

            hbuf = pp.tile([P, SHARD], tdt)
            x2buf = pp.tile([P, SHARD], bf)
            # layer-2 h' reuses hbuf: transform2 only starts after every
            # agg1 epilogue has read its hbuf slice (program order), and
            # Tile's tile-granular WAR ordering covers the reuse.
            h2buf = hbuf

            def transform(get_lhsT, w_tiles, out_sbuf, cc_dram):
                nkt = len(w_tiles)
                for t in range(NB):
                    hp = psp.tile([P, HID], f32, tag="hp")
                    for k in range(nkt):
                        nc.tensor.matmul(
                            out=hp[:], lhsT=get_lhsT(t, k), rhs=w_tiles[k],
                            start=(k == 0), stop=(k == nkt - 1),
                        )
                    sl = out_sbuf[:, t * P : (t + 1) * P]
                    nc.vector.tensor_scalar(
                        out=sl, in0=hp[:], scalar1=dinv_t[:, t : t + 1],
                        scalar2=None, op0=mybir.AluOpType.mult,
                    )
                    nc.sync.dma_start(
                        out=cc_dram[t * P : (t + 1) * P, :], in_=sl
                    )

            prep_counts = [0] * NQ
            # Tile assigns DMASW sem lanes round-robin over Pool DMA insts in
            # SCHEDULED order, and each sem is locked to one SWDGE queue — so
            # queue_num must track the emission counter mod NQ AND the
            # scheduler must not reorder gathers (nosync chain below).
            dma_count = [0]
            last_gather = [None]

            def aggregate(table, hsrc, bias_t, relu, z_dram, do_collective):
                # call tiles (per stream), chunk-indexed views
                lo_tiles = {}
                hi_tiles = {}

                def issue(stream, ci, prep):
                    calls, tiles, gp = (
                        (calls_lo, lo_tiles, gplo)
                        if stream == "lo"
                        else (calls_hi, hi_tiles, gphi)
                    )
                    at, k = calls[ci]
                    n = k * P
                    src = table[HI_BASE:, :] if stream == "hi" else table[:, :]
                    col = at if stream == "lo" else NLO + at
                    q = dma_count[0] % NQ
                    dma_count[0] += 1
                    kw = (
                        dict(prepare_only=True, sem=dma_sems[q]) if prep else {}
                    )
                    G = gp.tile([P, CALL_CHUNKS * HID], tdt, tag=f"g{stream}")
                    G3 = G[:].rearrange("p (c d) -> p c d", d=HID)
                    tiles[ci] = G3
                    g = nc.gpsimd.dma_gather(
                        G3[:, 0:k, :],
                        src,
                        idx_t[:, col * 8 : col * 8 + n // 16],
                        n, n, HID,
                        queue_num=q,
                        **kw,
                    )
                    if last_gather[0] is not None:
                        g.ins.add_nosync_dependencies_from(last_gather[0])
                    d = InstructionNameOrderedSet()
                    d.add(g.ins.name)
                    last_gather[0] = d
                    return q

                # prep the leading calls of each stream on the otherwise
                # idle Pool engine before the AllGather (ring holds 4
                # 1024-desc calls per queue -> up to 16 preps)
                npre_lo = min(8, len(calls_lo))
                npre_hi = min(4, len(calls_hi))
                prep_q = {}
                for ci in range(npre_lo):
                    q = issue("lo", ci, True)
                    prep_counts[q] += 1
                    prep_q[("lo", ci)] = q
                for ci in range(npre_hi):
                    q = issue("hi", ci, True)
                    prep_counts[q] += 1
                    prep_q[("hi", ci)] = q

                ag = do_collective()
                ag_dep = InstructionNameOrderedSet()
                ag_dep.add(ag.ins.name)
                qs_used = sorted(set(prep_q.values()))
                trig_dep = InstructionNameOrderedSet()
                for q in qs_used:
                    trig = nc.gpsimd.trigger_dma(count=None, queue_num=q)
                    trig.ins.add_sync_dependencies_from(ag_dep)
                    trig_dep.add(trig.ins.name)
                gate_deps = {}
                for q in qs_used:
                    gate_t = ep.tile([P, 4], f32, tag="gate")
                    g_i = nc.gpsimd.memset(gate_t[:], 0.0)
                    g_i._wait_ge(dma_sems[q], 16 * prep_counts[q])
                    g_i.ins.add_nosync_dependencies_from(trig_dep)
                    d = InstructionNameOrderedSet()
                    d.add(g_i.ins.name)
                    gate_deps[q] = d
                ungated = {k: prep_q[k] for k in prep_q}

                li = npre_lo
                hj = npre_hi

                for b in range(NB):
                    lo_end = int(lo0[b]) + int(C_lo[b])
                    hi_end = int(hi0[b]) + int(C_hi[b])
                    while li < len(calls_lo) and calls_lo[li][0] < lo_end:
                        issue("lo", li, False)
                        li += 1
                    while hj < len(calls_hi) and calls_hi[hj][0] < hi_end:
                        issue("hi", hj, False)
                        hj += 1
                    acc = psp.tile([P, HID], f32, tag="acc")
                    # +1 fold: the self-loop h'_own folds in via an identity
                    # matmul, so the epilogue is a single ScalarE activation
                    # (biases are zero for this problem).
                    nfold = int(C_lo[b]) + int(C_hi[b]) + 1
                    fi = 0
                    for stream, c0, cnt, coff, tiles in (
                        ("lo", int(lo0[b]), int(C_lo[b]), 0, lo_tiles),
                        ("hi", int(hi0[b]), int(C_hi[b]), NLO, hi_tiles),
                    ):
                        for i in range(cnt):
                            ch = c0 + i  # stream-local chunk index
                            col = coff + ch  # dmat column
                            ci = ch // CALL_CHUNKS
                            slot = ch % CALL_CHUNKS
                            S = sp.tile([P, P], tdt, tag="S")
                            nc.vector.tensor_scalar(
                                out=S[:], in0=iota_t[:],
                                scalar1=dm_t[:, col : col + 1],
                                scalar2=None,
                                op0=mybir.AluOpType.is_equal,
                            )
                            mm = nc.tensor.matmul(
                                out=acc[:], lhsT=S[:],
                                rhs=tiles[ci][:, slot, :],
                                start=(fi == 0), stop=False,
                            )
                            key = (stream, ci)
                            if key in ungated:
                                mm.ins.add_sync_dependencies_from(
                                    gate_deps[ungated.pop(key)]
                                )
                            fi += 1
                    nc.tensor.matmul(
                        out=acc[:], lhsT=ident_t[:],
                        rhs=hsrc[:, b * P : (b + 1) * P],
                        start=(fi == 0), stop=True,
                    )
                    if has_bias:
                        t2 = ep.tile([P, HID], f32, tag="t2")
                        nc.vector.scalar_tensor_tensor(
                            out=t2[:], in0=acc[:],
                            scalar=dinv_t[:, b : b + 1], in1=bias_t[:],
                            op0=mybir.AluOpType.mult,
                            op1=mybir.AluOpType.add,
                        )
                        if relu:
                            nc.scalar.activation(
                                out=x2buf[:, b * P : (b + 1) * P], in_=t2[:],
                                func=mybir.ActivationFunctionType.Relu,
                            )
                        else:
                            nc.sync.dma_start(
                                out=z_dram[b * P : (b + 1) * P, :], in_=t2[:]
                            )
                    elif relu:
                        nc.scalar.activation(
                            out=x2buf[:, b * P : (b + 1) * P], in_=acc[:],
                            func=mybir.ActivationFunctionType.Relu,
                            scale=dinv_t[:, b : b + 1],
                        )
                    else:
                        t2 = ep.tile([P, HID], f32, tag="t2")
                        nc.scalar.mul(
                            out=t2[:], in_=acc[:], mul=dinv_t[:, b : b + 1]
                        )
                        nc.sync.dma_start(
                            out=z_dram[b * P : (b + 1) * P, :], in_=t2[:]
                        )

            # layer 1
            xT_t = pp.tile([P, 2 * SHARD], bf)
            nc.sync.dma_start(out=xT_t[:], in_=x_in[:])
            xT3 = xT_t[:].rearrange("p (k n) -> p k n", k=2)
            transform(
                lambda t, k: xT3[:, k, t * P : (t + 1) * P],
                [w1_t[:, 0:HID], w1_t[:, HID : 2 * HID]], hbuf, cc1_in,
            )
            aggregate(
                table1, hbuf, b1_t, True, None,
                lambda: nc.gpsimd.collective_compute(
                    "AllGather", mybir.AluOpType.bypass, replica_groups=rg,
                    ins=[cc1_in[:]], outs=[table1[:]],
                ),
            )

            # layer 2
            def l2_lhsT(t, k):
                tp = psp.tile([P, P], bf, tag="tp")
                nc.tensor.transpose(
                    out=tp[:], in_=x2buf[:, t * P : (t + 1) * P],
                    identity=ident_t[:],
                )
                xT = xtp.tile([P, P], bf, tag="xT")
                nc.scalar.copy(out=xT[:], in_=tp[:])
                return xT[:]

            transform(l2_lhsT, [w2_t[:]], h2buf, cc2_in)
            aggregate(
                table2, h2buf, b2_t, False, z_out,
                lambda: nc.gpsimd.collective_compute(
                    "AllGather", mybir.AluOpType.bypass, replica_groups=rg,
                    ins=[cc2_in[:]], outs=[table2[:]],
                ),
            )

    nc.compile()
    return nc


def kernel(x, edge_index, W1, b1, W2, b2):
    global LAST_EXEC_NS, LAST_RESULT
    from concourse.bass_utils import run_bass_kernel_spmd

    trace = bool(int(os.environ.get("GCN_TRACE", "0")))
    if trace:
        try:  # NTFF profiling shim (axon images lack antenv.axon_hooks)
            _install_ntff_shim()
        except Exception:
            trace = False

    in_maps, meta = _host_prep(x, edge_index, W1, b1, W2, b2)
    nc = _build_program(meta)
    res = run_bass_kernel_spmd(
        nc, in_maps, core_ids=list(range(N_CORES)), trace=trace
    )
    LAST_EXEC_NS = res.exec_time_ns
    LAST_RESULT = res

    old_of_new = meta["old_of_new"]
    z = np.zeros((N_NODES, HID), np.float32)
    for c in range(N_CORES):
        zc = np.asarray(res.results[c]["z"])
        sel = old_of_new[c * SHARD : (c + 1) * SHARD]
        real = sel >= 0
        z[sel[real]] = zc[real]
    return z


def _install_ntff_shim():
    import contextlib
    import ctypes
    import sys
    import types

    if "antenv.axon_hooks" in sys.modules:
        return
    lib = ctypes.CDLL("/opt/axon/libaxon_pjrt.so")
    if not hasattr(lib, "axon_start_nrt_profile"):
        raise RuntimeError("no profile symbols")
    lib.axon_start_nrt_profile.argtypes = [
        ctypes.POINTER(ctypes.c_int64),
        ctypes.c_size_t,
    ]
    lib.axon_start_nrt_profile.restype = ctypes.c_int64
    lib.axon_stop_nrt_profile.argtypes = [ctypes.c_char_p]
    lib.axon_stop_nrt_profile.restype = ctypes.c_int64

    @contextlib.contextmanager
    def _hook(output_dir, device_ids):
        import jax

        jax.devices()
        if device_ids:
            ids = (ctypes.c_int64 * len(device_ids))(*device_ids)
            rc = lib.axon_start_nrt_profile(ids, len(device_ids))
        else:
            rc = lib.axon_start_nrt_profile(None, 0)
        if rc != 0:
            raise RuntimeError(f"axon_start_nrt_profile rc={rc}")
        try:
            yield
        finally:
            lib.axon_stop_nrt_profile(str(output_dir).encode())

    mod = types.ModuleType("antenv.axon_hooks")
    mod.get_axon_ntff_profile_hook = lambda: _hook
    mod.set_axon_ntff_profile_hook = lambda h: None
    sys.modules["antenv.axon_hooks"] = mod
    import antenv

    antenv.axon_hooks = mod


# revision 16
# speedup vs baseline: 1.7085x; 1.4173x over previous
"""2-layer GCN (GCNConv -> ReLU -> GCNConv) on 8 TRN2 NeuronCores.

Sharding: output nodes are split into 8 shards (one per core); edges are
partitioned by destination shard so each core owns the scatter-add for its
nodes. Hidden features of source nodes are exchanged with an on-device
AllGather between the per-shard transform and the aggregation.

Per-core pipeline (single SPMD Bass program, all cores identical; per-core
behavior comes from per-core input data):
  1. transform: h' = dinv * (x @ W1) for the own shard (TensorE, bf16
     operands / fp32 PSUM accumulate), host-pretransposed bf16 x.
  2. AllGather h' (bf16) -> full 50176-row table in each core's DRAM.
  3. aggregation: edges are pre-sorted by destination block (49 blocks of
     128 dst nodes per core, balanced by in-degree via a host-side node
     relabeling). Source rows are fetched with dma_gather (256B bf16 rows).
     dma_gather indices are int16, so edges are segregated into a "lo"
     (src < 32768) stream and a "hi" stream addressed from a shifted table
     base. Each stream is a flat sequence of 128-edge chunks (per dst
     block: lo chunks then hi chunks, zero-padded to chunk granularity);
     gather calls cover up to CALL_CHUNKS consecutive chunks regardless of
     block boundaries, so the 994ns/call SWDGE desc-gen overhead amortizes
     over 4096 indices (needs dynamic_dma_scratch_size=64KB: the SWDGE
     ring carveout is scratch/16 descriptors per queue). lo calls ride
     queues 0/1, hi calls queues 2/3. A one-hot matrix S (tensor_scalar
     iota == dmat column, bf16 so DVE runs in 2x mode) folds each chunk
     into the block's PSUM accumulator on TensorE (bf16).
  4. out = dinv * (agg + h'_own) + bias (the self-loop and symmetric
     normalization fold into two dinv scalings); ReLU on ScalarE.
  5. repeat 1-4 with W2/b2; z shard is DMA'd out and un-permuted on host.

Gather prefetch: the first gather call of each queue per layer is
desc-gen'd with prepare_only=True on the otherwise idle Q7 before the
AllGather and fired by per-queue trigger_dma right after it. The triggers
carry explicit sync deps on the collective (the deferred RAW cannot see a
writer emitted after the prep), and per-queue gpsimd memset "gate"
instructions carrying _wait_ge on the prep DMA semaphores give the prepped
calls' fold chains a hardware data gate (Tile's own consumer dep resolves
on the prep's desc-written tick, not DMA completion).
"""

import os

import numpy as np

P = 128
N_CORES = 8
N_NODES = 50000
IN_DIM = 256
HID = 128
NB = 49
SHARD = NB * P  # 6272
NPAD = N_CORES * SHARD  # 50176
HI_BASE = 32768
CALL_CHUNKS = 8  # 1024 indices per gather call (hard SWDGE ucode limit)
DMA_SCRATCH = 65536  # SWDGE ring carveout: 4096 descriptors per queue
GLO_BUFS = 12  # lo-stream call-tile pool depth
GHI_BUFS = 8  # hi-stream call-tile pool depth
SBATCH = 8  # chunks per batched one-hot build on DVE

LAST_EXEC_NS = None
LAST_RESULT = None


def _wrap16(flat, ncols):
    w = np.zeros((16, ncols), np.uint16)
    n = len(flat)
    w[np.arange(n) % 16, np.arange(n) // 16] = flat
    return np.tile(w, (8, 1)).view(np.int16)


def _host_prep(x, edge_index, W1, b1, W2, b2):
    src = np.asarray(edge_index[0], dtype=np.int64)
    dst = np.asarray(edge_index[1], dtype=np.int64)
    x = np.asarray(x, dtype=np.float32)

    indeg = np.bincount(dst, minlength=N_NODES)
    deg = indeg + 1.0
    dinv = (1.0 / np.sqrt(deg)).astype(np.float32)

    # per-shard relabeling: deal nodes (by in-degree desc) round-robin into
    # the 49 dst blocks so block edge counts are balanced across cores.
    old_shard = N_NODES // N_CORES
    new_of_old = np.empty(N_NODES, np.int64)
    old_of_new = np.full(NPAD, -1, np.int64)
    for c in range(N_CORES):
        olds = np.arange(c * old_shard, (c + 1) * old_shard)
        order = olds[np.argsort(-indeg[olds], kind="stable")]
        pos_in_block = np.arange(len(order)) // NB
        block = np.arange(len(order)) % NB
        news = c * SHARD + block * P + pos_in_block
        new_of_old[order] = news
        old_of_new[news] = order

    src_n = new_of_old[src]
    dst_n = new_of_old[dst]

    core_of_dst = dst_n // SHARD
    lo_lists = [[None] * NB for _ in range(N_CORES)]
    hi_lists = [[None] * NB for _ in range(N_CORES)]
    for c in range(N_CORES):
        m = core_of_dst == c
        s, d = src_n[m], dst_n[m] - c * SHARD
        b = d // P
        r = d % P
        hi = s >= HI_BASE
        for bb in range(NB):
            mb = b == bb
            mlo = mb & ~hi
            mhi = mb & hi
            lo_lists[c][bb] = (s[mlo], r[mlo])
            hi_lists[c][bb] = (s[mhi] - HI_BASE, r[mhi])

    # common (max-over-cores) chunk counts per block and region
    C_lo = np.zeros(NB, np.int64)
    C_hi = np.zeros(NB, np.int64)
    for b in range(NB):
        for c in range(N_CORES):
            C_lo[b] = max(C_lo[b], (len(lo_lists[c][b][0]) + P - 1) // P)
            C_hi[b] = max(C_hi[b], (len(hi_lists[c][b][0]) + P - 1) // P)
    lo0 = np.concatenate([[0], np.cumsum(C_lo)])  # [NB+1]
    hi0 = np.concatenate([[0], np.cumsum(C_hi)])
    NLO = int(lo0[-1])
    NHI = int(hi0[-1])
    NS = NLO + NHI

    # per-core flat index stream ([lo chunks][hi chunks]) + dst-row matrix
    idx_mats, d_mats = [], []
    for c in range(N_CORES):
        idx_flat = np.zeros(NS * P, np.int64)
        dloc = np.full((P, NS), -1.0, np.float32)
        for b in range(NB):
            for lists, c0 in ((lo_lists, lo0[b]), (hi_lists, NLO + hi0[b])):
                s, r = lists[c][b]
                n = len(s)
                base = int(c0) * P
                idx_flat[base : base + n] = s
                j = np.arange(n)
                dloc[j % P, int(c0) + j // P] = r
        idx_mats.append(_wrap16(idx_flat, NS * 8))
        d_mats.append(dloc)

    def mk_calls(total):
        calls = []
        at = 0
        while at < total:
            k = min(CALL_CHUNKS, total - at)
            calls.append((at, k))
            at += k
        return calls

    calls_lo = mk_calls(NLO)
    calls_hi = mk_calls(NHI)

    xs, dinvs = [], []
    for c in range(N_CORES):
        xc = np.zeros((SHARD, IN_DIM), np.float32)
        dc = np.ones((SHARD,), np.float32)
        sel = old_of_new[c * SHARD : (c + 1) * SHARD]
        real = sel >= 0
        xc[real] = x[sel[real]]
        dc[real] = dinv[sel[real]]
        dw = dc.reshape(NB, P).T.copy()
        xT = np.ascontiguousarray(xc.T.reshape(2, P, SHARD).transpose(1, 0, 2))
        xs.append(xT.reshape(P, 2 * SHARD))
        dinvs.append(dw)

    import ml_dtypes

    bf16 = ml_dtypes.bfloat16
    iota = np.tile(np.arange(P, dtype=np.float32)[None, :], (P, 1))
    ident = np.eye(P, dtype=np.float32).astype(bf16)
    b1r = np.tile(np.asarray(b1, np.float32)[None, :], (P, 1))
    b2r = np.tile(np.asarray(b2, np.float32)[None, :], (P, 1))

    in_maps = []
    for c in range(N_CORES):
        in_maps.append(
            {
                "x": xs[c].astype(bf16),
                "gidx": idx_mats[c],
                "dmat": d_mats[c],
                "dinv": dinvs[c],
                "w1": np.asarray(W1, np.float32).astype(bf16),
                "w2": np.asarray(W2, np.float32).astype(bf16),
                "b1r": b1r,
                "b2r": b2r,
                "iota": iota.astype(bf16),
                "ident": ident,
            }
        )

    meta = dict(
        C_lo=C_lo, C_hi=C_hi, lo0=lo0, hi0=hi0, NLO=NLO, NHI=NHI, NS=NS,
        calls_lo=calls_lo, calls_hi=calls_hi, old_of_new=old_of_new,
        has_bias=bool(np.any(np.asarray(b1)) or np.any(np.asarray(b2))),
    )
    return in_maps, meta


NQ = 4  # SWDGE queues: lo stream on 0/1, hi stream on 2/3


def _build_program(meta):
    import concourse.mybir as mybir
    import concourse.tile as tile
    from concourse import bacc
    from concourse._compat import get_trn_type
    from concourse.instruction_name_ordered_set import InstructionNameOrderedSet

    C_lo, C_hi = meta["C_lo"], meta["C_hi"]
    has_bias = meta["has_bias"]
    lo0, hi0 = meta["lo0"], meta["hi0"]
    NLO, NHI, NS = meta["NLO"], meta["NHI"], meta["NS"]
    calls_lo, calls_hi = meta["calls_lo"], meta["calls_hi"]
    f32 = mybir.dt.float32
    bf = mybir.dt.bfloat16
    tdt = bf

    nc = bacc.Bacc(
        get_trn_type() or "TRN2",
        num_swdge_queues=NQ,
        dynamic_dma_scratch_size=DMA_SCRATCH,
    )
    x_in = nc.dram_tensor("x", [P, 2 * SHARD], bf, kind="ExternalInput")
    gidx = nc.dram_tensor("gidx", [P, NS * 8], mybir.dt.int16, kind="ExternalInput")
    dmat = nc.dram_tensor("dmat", [P, NS], f32, kind="ExternalInput")
    dinv_in = nc.dram_tensor("dinv", [P, NB], f32, kind="ExternalInput")
    w1_in = nc.dram_tensor("w1", [IN_DIM, HID], bf, kind="ExternalInput")
    w2_in = nc.dram_tensor("w2", [HID, HID], bf, kind="ExternalInput")
    b1_in = nc.dram_tensor("b1r", [P, HID], f32, kind="ExternalInput")
    b2_in = nc.dram_tensor("b2r", [P, HID], f32, kind="ExternalInput")
    iota_in = nc.dram_tensor("iota", [P, P], bf, kind="ExternalInput")
    ident_in = nc.dram_tensor("ident", [P, P], bf, kind="ExternalInput")
    z_out = nc.dram_tensor("z", [SHARD, HID], f32, kind="ExternalOutput")

    cc1_in = nc.dram_tensor("cc1_in", [SHARD, HID], tdt)
    table1 = nc.dram_tensor("table1", [NPAD, HID], tdt, addr_space="Shared")
    cc2_in = nc.dram_tensor("cc2_in", [SHARD, HID], tdt)
    table2 = nc.dram_tensor("table2", [NPAD, HID], tdt, addr_space="Shared")

    rg = [list(range(N_CORES))]
    dma_sems = [nc.alloc_semaphore(f"gdma{q}") for q in range(NQ)]

    with tile.TileContext(nc) as tc:
        for _s in dma_sems:
            nc.gpsimd.sem_clear(_s)
        with (
            tc.tile_pool(name="persist", bufs=1) as pp,
            tc.tile_pool(name="xt", bufs=4) as xtp,
            tc.tile_pool(name="glo", bufs=GLO_BUFS) as gplo,
            tc.tile_pool(name="ghi", bufs=GHI_BUFS) as gphi,
            tc.tile_pool(name="s", bufs=8) as sp,
            tc.tile_pool(name="ep", bufs=4) as ep,
            tc.tile_pool(name="psum", bufs=2, space="PSUM") as psp,
        ):
            idx_t = pp.tile([P, NS * 8], mybir.dt.int16)
            nc.sync.dma_start(out=idx_t[:], in_=gidx[:])
            dm_t = pp.tile([P, NS], f32)
            nc.sync.dma_start(out=dm_t[:], in_=dmat[:])
            dinv_t = pp.tile([P, NB], f32)
            nc.sync.dma_start(out=dinv_t[:], in_=dinv_in[:])
            iota_t = pp.tile([P, P], bf)
            nc.sync.dma_start(out=iota_t[:], in_=iota_in[:])
            ident_t = pp.tile([P, P], bf)
            nc.sync.dma_start(out=ident_t[:], in_=ident_in[:])
            b1_t = pp.tile([P, HID], f32)
            nc.sync.dma_start(out=b1_t[:], in_=b1_in[:])
            b2_t = pp.tile([P, HID], f32)
            nc.sync.dma_start(out=b2_t[:], in_=b2_in[:])
            w1_t = pp.tile([P, 2 * HID], bf)
            nc.sync.dma_start(
                out=w1_t[:].rearrange("p (k h) -> p k h", k=2),
                in_=w1_in[:].rearrange("(k p) h -> p k h", p=P),
            )
            w2_t = pp.tile([P, HID], bf)
            nc.sync.dma_start(out=w2_t[:], in_=w2_in[:])

            hbuf = pp.tile([P, SHARD], tdt)
            x2buf = pp.tile([P, SHARD], bf)
            # layer-2 h' reuses hbuf: transform2 only starts after every
            # agg1 epilogue has read its hbuf slice (program order), and
            # Tile's tile-granular WAR ordering covers the reuse.
            h2buf = hbuf

            def transform(get_lhsT, w_tiles, out_sbuf, cc_dram):
                nkt = len(w_tiles)
                for t in range(NB):
                    hp = psp.tile([P, HID], f32, tag="hp")
                    for k in range(nkt):
                        nc.tensor.matmul(
                            out=hp[:], lhsT=get_lhsT(t, k), rhs=w_tiles[k],
                            start=(k == 0), stop=(k == nkt - 1),
                        )
                    sl = out_sbuf[:, t * P : (t + 1) * P]
                    nc.scalar.mul(out=sl, in_=hp[:], mul=dinv_t[:, t : t + 1])
                    nc.sync.dma_start(
                        out=cc_dram[t * P : (t + 1) * P, :], in_=sl
                    )

            prep_counts = [0] * NQ
            # Tile assigns DMASW sem lanes round-robin over Pool DMA insts in
            # SCHEDULED order, and each sem is locked to one SWDGE queue — so
            # queue_num must track the emission counter mod NQ AND the
            # scheduler must not reorder gathers (nosync chain below).
            dma_count = [0]
            last_gather = [None]

            def aggregate(table, hsrc, bias_t, relu, z_dram, do_collective):
                # call tiles (per stream), chunk-indexed views
                lo_tiles = {}
                hi_tiles = {}

                def issue(stream, ci, prep):
                    calls, tiles, gp = (
                        (calls_lo, lo_tiles, gplo)
                        if stream == "lo"
                        else (calls_hi, hi_tiles, gphi)
                    )
                    at, k = calls[ci]
                    n = k * P
                    src = table[HI_BASE:, :] if stream == "hi" else table[:, :]
                    col = at if stream == "lo" else NLO + at
                    q = dma_count[0] % NQ
                    dma_count[0] += 1
                    kw = (
                        dict(prepare_only=True, sem=dma_sems[q]) if prep else {}
                    )
                    G = gp.tile([P, CALL_CHUNKS * HID], tdt, tag=f"g{stream}")
                    G3 = G[:].rearrange("p (c d) -> p c d", d=HID)
                    tiles[ci] = G3
                    g = nc.gpsimd.dma_gather(
                        G3[:, 0:k, :],
                        src,
                        idx_t[:, col * 8 : col * 8 + n // 16],
                        n, n, HID,
                        queue_num=q,
                        **kw,
                    )
                    if last_gather[0] is not None:
                        g.ins.add_nosync_dependencies_from(last_gather[0])
                    d = InstructionNameOrderedSet()
                    d.add(g.ins.name)
                    last_gather[0] = d
                    return q

                # prep the leading calls of each stream on the otherwise
                # idle Pool engine before the AllGather (ring holds 4
                # 1024-desc calls per queue -> up to 16 preps)
                npre_lo = min(8, len(calls_lo))
                npre_hi = min(4, len(calls_hi))
                prep_q = {}
                for ci in range(npre_lo):
                    q = issue("lo", ci, True)
                    prep_counts[q] += 1
                    prep_q[("lo", ci)] = q
                for ci in range(npre_hi):
                    q = issue("hi", ci, True)
                    prep_counts[q] += 1
                    prep_q[("hi", ci)] = q

                ag = do_collective()
                ag_dep = InstructionNameOrderedSet()
                ag_dep.add(ag.ins.name)
                qs_used = sorted(set(prep_q.values()))
                trig_dep = InstructionNameOrderedSet()
                for q in qs_used:
                    trig = nc.gpsimd.trigger_dma(count=None, queue_num=q)
                    trig.ins.add_sync_dependencies_from(ag_dep)
                    trig_dep.add(trig.ins.name)
                gate_deps = {}
                for q in qs_used:
                    gate_t = ep.tile([P, 4], f32, tag="gate")
                    g_i = nc.gpsimd.memset(gate_t[:], 0.0)
                    g_i._wait_ge(dma_sems[q], 16 * prep_counts[q])
                    g_i.ins.add_nosync_dependencies_from(trig_dep)
                    d = InstructionNameOrderedSet()
                    d.add(g_i.ins.name)
                    gate_deps[q] = d
                ungated = {k: prep_q[k] for k in prep_q}

                li = npre_lo
                hj = npre_hi

                for b in range(NB):
                    lo_end = int(lo0[b]) + int(C_lo[b])
                    hi_end = int(hi0[b]) + int(C_hi[b])
                    while li < len(calls_lo) and calls_lo[li][0] < lo_end:
                        issue("lo", li, False)
                        li += 1
                    while hj < len(calls_hi) and calls_hi[hj][0] < hi_end:
                        issue("hi", hj, False)
                        hj += 1
                    acc = psp.tile([P, HID], f32, tag="acc")
                    # +1 fold: the self-loop h'_own folds in via an identity
                    # matmul, so the epilogue is a single ScalarE activation
                    # (biases are zero for this problem).
                    nfold = int(C_lo[b]) + int(C_hi[b]) + 1
                    fi = 0
                    for stream, c0, cnt, coff, tiles in (
                        ("lo", int(lo0[b]), int(C_lo[b]), 0, lo_tiles),
                        ("hi", int(hi0[b]), int(C_hi[b]), NLO, hi_tiles),
                    ):
                        for j0 in range(0, cnt, SBATCH):
                            nb = min(SBATCH, cnt - j0)
                            col = coff + c0 + j0
                            SB = sp.tile([P, SBATCH * P], tdt, tag="S")
                            S3 = SB[:].rearrange("p (k d) -> p k d", d=P)
                            nc.vector.tensor_tensor(
                                out=S3[:, 0:nb, :],
                                in0=iota_t[:]
                                .rearrange("p (k d) -> p k d", k=1)
                                .to_broadcast([P, nb, P]),
                                in1=dm_t[:, col : col + nb].to_broadcast(
                                    [P, nb, P]
                                ),
                                op=mybir.AluOpType.is_equal,
                            )
                            for i in range(j0, j0 + nb):
                                ch = c0 + i  # stream-local chunk index
                                ci = ch // CALL_CHUNKS
                                slot = ch % CALL_CHUNKS
                                mm = nc.tensor.matmul(
                                    out=acc[:], lhsT=S3[:, i - j0, :],
                                    rhs=tiles[ci][:, slot, :],
                                    start=(fi == 0), stop=False,
                                )
                                key = (stream, ci)
                                if key in ungated:
                                    mm.ins.add_sync_dependencies_from(
                                        gate_deps[ungated.pop(key)]
                                    )
                                fi += 1
                    nc.tensor.matmul(
                        out=acc[:], lhsT=ident_t[:],
                        rhs=hsrc[:, b * P : (b + 1) * P],
                        start=(fi == 0), stop=True,
                    )
                    if has_bias:
                        t2 = ep.tile([P, HID], f32, tag="t2")
                        nc.vector.scalar_tensor_tensor(
                            out=t2[:], in0=acc[:],
                            scalar=dinv_t[:, b : b + 1], in1=bias_t[:],
                            op0=mybir.AluOpType.mult,
                            op1=mybir.AluOpType.add,
                        )
                        if relu:
                            nc.scalar.activation(
                                out=x2buf[:, b * P : (b + 1) * P], in_=t2[:],
                                func=mybir.ActivationFunctionType.Relu,
                            )
                        else:
                            nc.sync.dma_start(
                                out=z_dram[b * P : (b + 1) * P, :], in_=t2[:]
                            )
                    elif relu:
                        nc.scalar.activation(
                            out=x2buf[:, b * P : (b + 1) * P], in_=acc[:],
                            func=mybir.ActivationFunctionType.Relu,
                            scale=dinv_t[:, b : b + 1],
                        )
                    else:
                        t2 = ep.tile([P, HID], f32, tag="t2")
                        nc.scalar.mul(
                            out=t2[:], in_=acc[:], mul=dinv_t[:, b : b + 1]
                        )
                        nc.sync.dma_start(
                            out=z_dram[b * P : (b + 1) * P, :], in_=t2[:]
                        )

            # layer 1
            xT_t = pp.tile([P, 2 * SHARD], bf)
            nc.sync.dma_start(out=xT_t[:], in_=x_in[:])
            xT3 = xT_t[:].rearrange("p (k n) -> p k n", k=2)
            transform(
                lambda t, k: xT3[:, k, t * P : (t + 1) * P],
                [w1_t[:, 0:HID], w1_t[:, HID : 2 * HID]], hbuf, cc1_in,
            )
            aggregate(
                table1, hbuf, b1_t, True, None,
                lambda: nc.gpsimd.collective_compute(
                    "AllGather", mybir.AluOpType.bypass, replica_groups=rg,
                    ins=[cc1_in[:]], outs=[table1[:]],
                ),
            )

            # layer 2
            def l2_lhsT(t, k):
                tp = psp.tile([P, P], bf, tag="tp")
                nc.tensor.transpose(
                    out=tp[:], in_=x2buf[:, t * P : (t + 1) * P],
                    identity=ident_t[:],
                )
                xT = xtp.tile([P, P], bf, tag="xT")
                nc.scalar.copy(out=xT[:], in_=tp[:])
                return xT[:]

            transform(l2_lhsT, [w2_t[:]], h2buf, cc2_in)
            aggregate(
                table2, h2buf, b2_t, False, z_out,
                lambda: nc.gpsimd.collective_compute(
                    "AllGather", mybir.AluOpType.bypass, replica_groups=rg,
                    ins=[cc2_in[:]], outs=[table2[:]],
                ),
            )

    nc.compile()
    return nc


def kernel(x, edge_index, W1, b1, W2, b2):
    global LAST_EXEC_NS, LAST_RESULT
    from concourse.bass_utils import run_bass_kernel_spmd

    trace = bool(int(os.environ.get("GCN_TRACE", "0")))
    if trace:
        try:  # NTFF profiling shim (axon images lack antenv.axon_hooks)
            _install_ntff_shim()
        except Exception:
            trace = False

    in_maps, meta = _host_prep(x, edge_index, W1, b1, W2, b2)
    nc = _build_program(meta)
    res = run_bass_kernel_spmd(
        nc, in_maps, core_ids=list(range(N_CORES)), trace=trace
    )
    LAST_EXEC_NS = res.exec_time_ns
    LAST_RESULT = res

    old_of_new = meta["old_of_new"]
    z = np.zeros((N_NODES, HID), np.float32)
    for c in range(N_CORES):
        zc = np.asarray(res.results[c]["z"])
        sel = old_of_new[c * SHARD : (c + 1) * SHARD]
        real = sel >= 0
        z[sel[real]] = zc[real]
    return z


def _install_ntff_shim():
    import contextlib
    import ctypes
    import sys
    import types

    if "antenv.axon_hooks" in sys.modules:
        return
    lib = ctypes.CDLL("/opt/axon/libaxon_pjrt.so")
    if not hasattr(lib, "axon_start_nrt_profile"):
        raise RuntimeError("no profile symbols")
    lib.axon_start_nrt_profile.argtypes = [
        ctypes.POINTER(ctypes.c_int64),
        ctypes.c_size_t,
    ]
    lib.axon_start_nrt_profile.restype = ctypes.c_int64
    lib.axon_stop_nrt_profile.argtypes = [ctypes.c_char_p]
    lib.axon_stop_nrt_profile.restype = ctypes.c_int64

    @contextlib.contextmanager
    def _hook(output_dir, device_ids):
        import jax

        jax.devices()
        if device_ids:
            ids = (ctypes.c_int64 * len(device_ids))(*device_ids)
            rc = lib.axon_start_nrt_profile(ids, len(device_ids))
        else:
            rc = lib.axon_start_nrt_profile(None, 0)
        if rc != 0:
            raise RuntimeError(f"axon_start_nrt_profile rc={rc}")
        try:
            yield
        finally:
            lib.axon_stop_nrt_profile(str(output_dir).encode())

    mod = types.ModuleType("antenv.axon_hooks")
    mod.get_axon_ntff_profile_hook = lambda: _hook
    mod.set_axon_ntff_profile_hook = lambda h: None
    sys.modules["antenv.axon_hooks"] = mod
    import antenv

    antenv.axon_hooks = mod
